# revision 16
# baseline (speedup 1.0000x reference)
"""ChannelAttentionModule Trainium2 kernel.

Reference computation (B=128, C=704, L=1024, G=11 groups of GW=64 channels):
    y_avg = mean(x, -1); y_max = max(x, -1)                      # [B, C]
    gate  = sigmoid(mlp(y_avg) + mlp(y_max))                     # [B, C]
    out   = x * gate[:, :, None]
where mlp is a per-group linear pair (W1[g]: 64x16, W2[g]: 16x64) with NO
nonlinearity between them, so mlp(a) + mlp(b) = a @ Wc + b @ Wc with
Wc[g] = W1[g] @ W2[g] (64x64), and mean = sum/L is folded in on DVE.

Sharding: data-parallel on batch across 8 cores (16 batches/core). Two
consecutive batches = 2*704 = 1408 rows = 11 row-tiles of 128, and each
64-row half-tile is one complete (batch, group) channel block. Blocks are
gathered PHASE-major: block t holds row-tile phase t (of 11) from all 8
batch-pairs, so every [128, 8, 1024] block shares ONE 128x128 block-diagonal
weight:
    load 4MB block -> reduce_sum + reduce_max over the whole block (2 DVE
    instrs) -> combine s/L+m (DVE) -> one matmul [128,128]x[128,8] (PE) ->
    sigmoid from PSUM (ACT) -> 8 per-pair scaled copies to an fp16 tile
    (ACT) -> store 2MB fp16.
fp16 stores halve write traffic (output quantization ~3e-4 rel, gate 2e-2);
the host upcasts to fp32. The last block is split into 4 sub-blocks to
shorten the pipeline drain.
"""

import os
import sys

import numpy as np

for _p in ("/opt/trn_rl_repo", "/root/.axon_site/_ro/trn_rl_repo"):
    if os.path.isdir(_p) and _p not in sys.path:
        sys.path.insert(0, _p)

import concourse.bacc as bacc
import concourse.bass as bass
import concourse.tile as tile
from concourse import mybir
from concourse.bass_utils import run_bass_kernel_spmd

B, C, L = 128, 704, 1024
G, GW = 11, 64
NCORES = 8
BPC = B // NCORES            # batches per core = 16
NPAIRS = BPC // 2            # 8
PAIR_ROWS = 2 * C            # 1408
NTILES = PAIR_ROWS // 128    # 11
ROWS = BPC * C               # 11264
F32 = mybir.dt.float32
F16 = mybir.dt.float16

_PROGRAM = None


def _build_program_v3(
    npairs=NPAIRS,
    xbufs=4,
    obufs=3,
    sbufs=8,
    split=1,
    last_split=4,
    store_q="sync",
    out16=True,
    dve_own=True,
    sig_psum=True,
    asum=3,
    gsum=0,
    dmul=0,
    w_after=True,
    junk16=False,
    jbufs=2,
    ssplit=1,
    ldmul=0,
):
    # Phase-major blocks: block t = [128, npairs, L] where slice a is
    # row-tile phase t of batch-pair a; all slices share weight t. The last
    # block is processed in `last_split` sub-blocks so the final
    # load->reduce->gate->scale->store chain (the pipeline drain) is short.
    nc = bacc.Bacc(None)
    rows = npairs * PAIR_ROWS
    odt = F16 if out16 else F32
    x = nc.declare_dram_parameter("x", [rows, L], F32, isOutput=False)
    w = nc.declare_dram_parameter("W", [128, NTILES * 128], F32, isOutput=False)
    out = nc.declare_dram_parameter("out", [rows, L], odt, isOutput=True)
    xr = x[:, :].rearrange("(a t p) l -> t p a l", a=npairs, t=NTILES, p=128)
    outr = out[:, :].rearrange("(a t p) l -> t p a l", a=npairs, t=NTILES, p=128)

    def subs_for(t):
        k = last_split if t == NTILES - 1 else split
        base = npairs // k
        return [(i * base, base) for i in range(k)]

    with tile.TileContext(nc) as tc:
        with (
            tc.tile_pool(name="singles", bufs=1) as singles,
            tc.tile_pool(name="xp", bufs=xbufs) as xp,
            tc.tile_pool(name="op", bufs=obufs) as op,
            tc.tile_pool(name="small", bufs=sbufs) as small,
            tc.tile_pool(name="psum", bufs=8, space=bass.MemorySpace.PSUM) as psums,
            tc.tile_pool(name="junkp", bufs=2) as junkp,
        ):

            def load_weights():
                if dve_own:
                    wt_raw = singles.tile([128, NTILES * 128], F32)
                    nc.sync.dma_start(out=wt_raw, in_=w[:, :])
                    wt = singles.tile([128, NTILES * 128], F32)
                    nc.vector.tensor_copy(out=wt, in_=wt_raw)
                else:
                    wt = singles.tile([128, NTILES * 128], F32)
                    nc.sync.dma_start(out=wt, in_=w[:, :])
                return wt

            def load_block(t):
                xt = xp.tile([128, npairs, L], F32, tag="x")
                for a0, cnt in subs_for(t):
                    nc.sync.dma_start(
                        out=xt[:, a0 : a0 + cnt, :], in_=xr[t][:, a0 : a0 + cnt, :]
                    )
                return xt

            if w_after:
                xt0 = load_block(0)
                wt = load_weights()
            else:
                wt = load_weights()
                xt0 = None

            for t in range(NTILES):
                pieces = subs_for(t)
                xt = xt0 if (t == 0 and xt0 is not None) else load_block(t)
                for a0, cnt in pieces:
                    xs = xt[:, a0 : a0 + cnt, :]
                    s = small.tile([128, cnt], F32, tag="s")
                    m = small.tile([128, cnt], F32, tag="m")
                    # Split this piece's pair-sums across ACT (activation
                    # Copy with accum_out), GPSIMD (tensor_scalar with
                    # accum_out), and DVE (reduce_sum) so every engine stays
                    # under the per-block DMA budget.
                    na = min((asum * cnt + npairs - 1) // npairs, cnt)
                    ng = min((gsum * cnt + npairs - 1) // npairs, cnt - na)
                    jdt = F16 if junk16 else F32
                    for a in range(na):
                        junk = junkp.tile([128, L], jdt, tag="ja", bufs=jbufs)
                        nc.scalar.activation(
                            out=junk, in_=xs[:, a, :],
                            func=mybir.ActivationFunctionType.Copy,
                            accum_out=s[:, a : a + 1],
                        )
                    for a in range(na, na + ng):
                        junk = junkp.tile([128, L], jdt, tag="jg", bufs=jbufs)
                        nc.gpsimd.tensor_scalar(
                            out=junk, in0=xs[:, a, :], scalar1=1.0, scalar2=0.0,
                            op0=mybir.AluOpType.mult, op1=mybir.AluOpType.add,
                            accum_out=s[:, a : a + 1],
                        )
                    if na + ng < cnt:
                        nc.vector.reduce_sum(
                            out=s[:, na + ng : cnt], in_=xs[:, na + ng : cnt, :],
                            axis=mybir.AxisListType.X,
                        )
                    nc.vector.reduce_max(out=m, in_=xs, axis=mybir.AxisListType.X)
                    comb = small.tile([128, cnt], F32, tag="c")
                    nc.vector.scalar_tensor_tensor(
                        out=comb, in0=s, scalar=1.0 / L, in1=m,
                        op0=mybir.AluOpType.mult, op1=mybir.AluOpType.add,
                    )
                    pc = psums.tile([128, cnt], F32)
                    nc.tensor.matmul(
                        pc, wt[:, t * 128 : (t + 1) * 128], comb,
                        start=True, stop=True,
                    )
                    if sig_psum:
                        sig_in = pc
                    else:
                        gsb = small.tile([128, cnt], F32, tag="gs")
                        nc.vector.tensor_copy(out=gsb, in_=pc)
                        sig_in = gsb
                    gate = small.tile([128, cnt], F32, tag="g")
                    nc.scalar.activation(
                        out=gate, in_=sig_in, func=mybir.ActivationFunctionType.Sigmoid
                    )
                    ot = op.tile([128, cnt, L], odt, tag="o")
                    # In the drain (last block) DVE is otherwise idle, so
                    # split the scale-muls across ACT and DVE there even when
                    # steady-state dmul is 0.
                    dm = ldmul if t == NTILES - 1 else dmul
                    nd = min((dm * cnt + npairs - 1) // npairs, cnt)
                    if store_q == "gps":
                        seng = nc.gpsimd
                    elif store_q == "act":
                        seng = nc.scalar
                    else:
                        seng = nc.sync
                    # Issue the store in `ssplit` pieces so the DMA ring gets
                    # fed partway through the serial mul chain.
                    nsp = ssplit if cnt % ssplit == 0 else 1
                    spc = cnt // nsp
                    for a in range(cnt):
                        if a < cnt - nd:
                            nc.scalar.mul(
                                out=ot[:, a, :], in_=xs[:, a, :],
                                mul=gate[:, a : a + 1],
                            )
                        else:
                            nc.vector.tensor_scalar_mul(
                                out=ot[:, a, :], in0=xs[:, a, :],
                                scalar1=gate[:, a : a + 1],
                            )
                        if (a + 1) % spc == 0:
                            b0 = a + 1 - spc
                            seng.dma_start(
                                out=outr[t][:, a0 + b0 : a0 + a + 1, :],
                                in_=ot[:, b0 : a + 1, :],
                            )
    if not nc.is_finalized():
        nc.finalize()
    return nc


def _pack_weights(W1, W2):
    # Wc[g] = W1[g] @ W2[g]; phase t holds channel blocks 2t (partitions
    # 0:64) and 2t+1 (partitions 64:128); block k -> group k % 11. The 1/L
    # mean scale is applied on DVE when combining sum+max, so weights are
    # unscaled.
    Wc = np.einsum(
        "gch,ghd->gcd", W1.astype(np.float64), W2.astype(np.float64)
    ).astype(np.float32)
    wpk = np.zeros((128, NTILES, 128), np.float32)
    for t in range(NTILES):
        gt, gb = (2 * t) % G, (2 * t + 1) % G
        wpk[0:64, t, 0:64] = Wc[gt]
        wpk[64:128, t, 64:128] = Wc[gb]
    return wpk.reshape(128, NTILES * 128)


def _get_program():
    global _PROGRAM
    if _PROGRAM is None:
        _PROGRAM = _build_program_v3()
    return _PROGRAM


_PACK = None


def run(x, W1, W2, trace=False, **kwargs):
    nc = _get_program()
    pack = _PACK if _PACK is not None else _pack_weights
    wpk = pack(np.asarray(W1), np.asarray(W2))
    xs = np.ascontiguousarray(x).reshape(NCORES, ROWS, L)
    in_maps = [{"x": xs[i], "W": wpk} for i in range(NCORES)]
    res = run_bass_kernel_spmd(
        nc, in_maps, core_ids=list(range(NCORES)), trace=trace, **kwargs
    )
    out = np.empty((NCORES, ROWS, L), np.float32)
    for i in range(NCORES):
        out[i] = res.results[i]["out"]
    return out.reshape(B, C, L), res


def kernel(x, W1, W2):
    out, _ = run(x, W1, W2)
    return out


# revision 19
# speedup vs baseline: 1.1753x; 1.1753x over previous
"""ChannelAttentionModule Trainium2 kernel.

Reference computation (B=128, C=704, L=1024, G=11 groups of GW=64 channels):
    y_avg = mean(x, -1); y_max = max(x, -1)                      # [B, C]
    gate  = sigmoid(mlp(y_avg) + mlp(y_max))                     # [B, C]
    out   = x * gate[:, :, None]
where mlp is a per-group linear pair (W1[g]: 64x16, W2[g]: 16x64) with NO
nonlinearity between them, so mlp(a) + mlp(b) = a @ Wc + b @ Wc with
Wc[g] = W1[g] @ W2[g] (64x64), and mean = sum/L is folded in on DVE.

Sharding: data-parallel on batch across 8 cores (16 batches/core). Two
consecutive batches = 2*704 = 1408 rows = 11 row-tiles of 128, and each
64-row half-tile is one complete (batch, group) channel block. Blocks are
gathered PHASE-major: block t holds row-tile phase t (of 11) from all 8
batch-pairs, so every [128, 8, 1024] block shares ONE 128x128 block-diagonal
weight:
    load 4MB block -> reduce_sum + reduce_max over the whole block (2 DVE
    instrs) -> combine s/L+m (DVE) -> one matmul [128,128]x[128,8] (PE) ->
    sigmoid from PSUM (ACT) -> 8 per-pair scaled copies to an fp16 tile
    (ACT) -> store 2MB fp16.
fp16 stores halve write traffic (output quantization ~3e-4 rel, gate 2e-2);
the host upcasts to fp32. The last block is split into 4 sub-blocks to
shorten the pipeline drain.
"""

import os
import sys

import numpy as np

for _p in ("/opt/trn_rl_repo", "/root/.axon_site/_ro/trn_rl_repo"):
    if os.path.isdir(_p) and _p not in sys.path:
        sys.path.insert(0, _p)

import concourse.bacc as bacc
import concourse.bass as bass
import concourse.tile as tile
from concourse import mybir
from concourse.bass_utils import run_bass_kernel_spmd

B, C, L = 128, 704, 1024
G, GW = 11, 64
NCORES = 8
BPC = B // NCORES            # batches per core = 16
NPAIRS = BPC // 2            # 8
PAIR_ROWS = 2 * C            # 1408
NTILES = PAIR_ROWS // 128    # 11
ROWS = BPC * C               # 11264
F32 = mybir.dt.float32
F16 = mybir.dt.float16

_PROGRAM = None


def _build_program_v3(
    npairs=NPAIRS,
    xbufs=4,
    obufs=3,
    sbufs=8,
    split=1,
    last_split=4,
    store_q="sync",
    out16=True,
    dve_own=True,
    sig_psum=True,
    asum=3,
    gsum=0,
    dmul=0,
    w_after=True,
    junk16=False,
    jbufs=2,
    ssplit=1,
    ldmul=0,
    pipe=False,
):
    # Phase-major blocks: block t = [128, npairs, L] where slice a is
    # row-tile phase t of batch-pair a; all slices share weight t. The last
    # block is processed in `last_split` sub-blocks so the final
    # load->reduce->gate->scale->store chain (the pipeline drain) is short.
    nc = bacc.Bacc(None)
    rows = npairs * PAIR_ROWS
    odt = F16 if out16 else F32
    x = nc.declare_dram_parameter("x", [rows, L], F32, isOutput=False)
    w = nc.declare_dram_parameter("W", [128, NTILES * 128], F32, isOutput=False)
    out = nc.declare_dram_parameter("out", [rows, L], odt, isOutput=True)
    xr = x[:, :].rearrange("(a t p) l -> t p a l", a=npairs, t=NTILES, p=128)
    outr = out[:, :].rearrange("(a t p) l -> t p a l", a=npairs, t=NTILES, p=128)

    def subs_for(t):
        k = last_split if t == NTILES - 1 else split
        base = npairs // k
        return [(i * base, base) for i in range(k)]

    with tile.TileContext(nc) as tc:
        with (
            tc.tile_pool(name="singles", bufs=1) as singles,
            tc.tile_pool(name="xp", bufs=xbufs) as xp,
            tc.tile_pool(name="op", bufs=obufs) as op,
            tc.tile_pool(name="small", bufs=sbufs) as small,
            tc.tile_pool(name="psum", bufs=8, space=bass.MemorySpace.PSUM) as psums,
            tc.tile_pool(name="junkp", bufs=2) as junkp,
        ):

            def load_weights():
                if dve_own:
                    wt_raw = singles.tile([128, NTILES * 128], F32)
                    nc.sync.dma_start(out=wt_raw, in_=w[:, :])
                    wt = singles.tile([128, NTILES * 128], F32)
                    nc.vector.tensor_copy(out=wt, in_=wt_raw)
                else:
                    wt = singles.tile([128, NTILES * 128], F32)
                    nc.sync.dma_start(out=wt, in_=w[:, :])
                return wt

            def load_block(t):
                xt = xp.tile([128, npairs, L], F32, tag="x")
                for a0, cnt in subs_for(t):
                    nc.sync.dma_start(
                        out=xt[:, a0 : a0 + cnt, :], in_=xr[t][:, a0 : a0 + cnt, :]
                    )
                return xt

            if w_after:
                xt0 = load_block(0)
                wt = load_weights()
            else:
                wt = load_weights()
                xt0 = None

            def sums_part(t, xt, a0, cnt):
                # Split this piece's pair-sums across ACT (activation Copy
                # with accum_out) and DVE (reduce_sum) so every engine stays
                # under the per-block DMA budget.
                xs = xt[:, a0 : a0 + cnt, :]
                s = small.tile([128, cnt], F32, tag="s")
                m = small.tile([128, cnt], F32, tag="m")
                na = min((asum * cnt + npairs - 1) // npairs, cnt)
                ng = min((gsum * cnt + npairs - 1) // npairs, cnt - na)
                jdt = F16 if junk16 else F32
                for a in range(na):
                    junk = junkp.tile([128, L], jdt, tag="ja", bufs=jbufs)
                    nc.scalar.activation(
                        out=junk, in_=xs[:, a, :],
                        func=mybir.ActivationFunctionType.Copy,
                        accum_out=s[:, a : a + 1],
                    )
                for a in range(na, na + ng):
                    junk = junkp.tile([128, L], jdt, tag="jg", bufs=jbufs)
                    nc.gpsimd.tensor_scalar(
                        out=junk, in0=xs[:, a, :], scalar1=1.0, scalar2=0.0,
                        op0=mybir.AluOpType.mult, op1=mybir.AluOpType.add,
                        accum_out=s[:, a : a + 1],
                    )
                if na + ng < cnt:
                    nc.vector.reduce_sum(
                        out=s[:, na + ng : cnt], in_=xs[:, na + ng : cnt, :],
                        axis=mybir.AxisListType.X,
                    )
                nc.vector.reduce_max(out=m, in_=xs, axis=mybir.AxisListType.X)
                return s, m

            def gate_part(t, s, m, cnt):
                comb = small.tile([128, cnt], F32, tag="c")
                nc.vector.scalar_tensor_tensor(
                    out=comb, in0=s, scalar=1.0 / L, in1=m,
                    op0=mybir.AluOpType.mult, op1=mybir.AluOpType.add,
                )
                pc = psums.tile([128, cnt], F32)
                nc.tensor.matmul(
                    pc, wt[:, t * 128 : (t + 1) * 128], comb,
                    start=True, stop=True,
                )
                if sig_psum:
                    sig_in = pc
                else:
                    gsb = small.tile([128, cnt], F32, tag="gs")
                    nc.vector.tensor_copy(out=gsb, in_=pc)
                    sig_in = gsb
                gate = small.tile([128, cnt], F32, tag="g")
                nc.scalar.activation(
                    out=gate, in_=sig_in,
                    func=mybir.ActivationFunctionType.Sigmoid,
                )
                return gate

            def muls_store(t, xt, a0, cnt, gate):
                xs = xt[:, a0 : a0 + cnt, :]
                ot = op.tile([128, cnt, L], odt, tag="o")
                # In the drain (last block) DVE is otherwise idle, so the
                # scale-muls can split across ACT and DVE there even when
                # steady-state dmul is 0.
                dm = ldmul if t == NTILES - 1 else dmul
                nd = min((dm * cnt + npairs - 1) // npairs, cnt)
                if store_q == "gps":
                    seng = nc.gpsimd
                elif store_q == "act":
                    seng = nc.scalar
                else:
                    seng = nc.sync
                nsp = ssplit if cnt % ssplit == 0 else 1
                spc = cnt // nsp
                for a in range(cnt):
                    if a < cnt - nd:
                        nc.scalar.mul(
                            out=ot[:, a, :], in_=xs[:, a, :],
                            mul=gate[:, a : a + 1],
                        )
                    else:
                        nc.vector.tensor_scalar_mul(
                            out=ot[:, a, :], in0=xs[:, a, :],
                            scalar1=gate[:, a : a + 1],
                        )
                    if (a + 1) % spc == 0:
                        b0 = a + 1 - spc
                        seng.dma_start(
                            out=outr[t][:, a0 + b0 : a0 + a + 1, :],
                            in_=ot[:, b0 : a + 1, :],
                        )

            if not pipe:
                for t in range(NTILES):
                    xt = xt0 if (t == 0 and xt0 is not None) else load_block(t)
                    for a0, cnt in subs_for(t):
                        s, m = sums_part(t, xt, a0, cnt)
                        gate = gate_part(t, s, m, cnt)
                        muls_store(t, xt, a0, cnt, gate)
            else:
                # Software pipeline: block t's ACT-sums are emitted BEFORE
                # block t-1's mul chain in ACT program order, so the next
                # block's stats don't queue behind the previous block's
                # 8-deep serial muls; the mul+store stage trails one block.
                prev = None
                for t in range(NTILES):
                    xt = xt0 if (t == 0 and xt0 is not None) else load_block(t)
                    pieces = subs_for(t)
                    sm = [sums_part(t, xt, a0, cnt) for a0, cnt in pieces]
                    if prev is not None:
                        for item in prev:
                            muls_store(*item)
                    prev = []
                    for (a0, cnt), (s, m) in zip(pieces, sm):
                        gate = gate_part(t, s, m, cnt)
                        prev.append((t, xt, a0, cnt, gate))
                for item in prev:
                    muls_store(*item)
    if not nc.is_finalized():
        nc.finalize()
    return nc


def _pack_weights(W1, W2):
    # Wc[g] = W1[g] @ W2[g]; phase t holds channel blocks 2t (partitions
    # 0:64) and 2t+1 (partitions 64:128); block k -> group k % 11. The 1/L
    # mean scale is applied on DVE when combining sum+max, so weights are
    # unscaled.
    Wc = np.einsum(
        "gch,ghd->gcd", W1.astype(np.float64), W2.astype(np.float64)
    ).astype(np.float32)
    wpk = np.zeros((128, NTILES, 128), np.float32)
    for t in range(NTILES):
        gt, gb = (2 * t) % G, (2 * t + 1) % G
        wpk[0:64, t, 0:64] = Wc[gt]
        wpk[64:128, t, 64:128] = Wc[gb]
    return wpk.reshape(128, NTILES * 128)


def _get_program():
    global _PROGRAM
    if _PROGRAM is None:
        _PROGRAM = _build_program_v3()
    return _PROGRAM


_PACK = None


def run(x, W1, W2, trace=False, **kwargs):
    nc = _get_program()
    pack = _PACK if _PACK is not None else _pack_weights
    wpk = pack(np.asarray(W1), np.asarray(W2))
    xs = np.ascontiguousarray(x).reshape(NCORES, ROWS, L)
    in_maps = [{"x": xs[i], "W": wpk} for i in range(NCORES)]
    res = run_bass_kernel_spmd(
        nc, in_maps, core_ids=list(range(NCORES)), trace=trace, **kwargs
    )
    out = np.empty((NCORES, ROWS, L), np.float32)
    for i in range(NCORES):
        out[i] = res.results[i]["out"]
    return out.reshape(B, C, L), res


def kernel(x, W1, W2):
    out, _ = run(x, W1, W2)
    return out
